# revision 7
# baseline (speedup 1.0000x reference)
"""Llama attention layer on 8 trn2 NeuronCores.

Sharding: data-parallel over batch (2) x tensor-parallel over head groups (4).
Each core handles one batch element and 8 of 32 heads (Wq/Wk/Wv column-shard,
Wo row-shard); host sums the 4 partial outputs per batch element.

Device layout notes:
 - Everything transposed: x.T, Q.T/K.T [head_dim(part), seq], V [seq(part), d].
 - scores_T[k,q] = K_T.T-free matmul (lhsT=K_T tile, rhs=Q_T) -> PSUM.
 - softmax without max-subtraction (scores are O(5), exp is safe in fp32);
   mask applied multiplicatively as exp(mask) in {0,1}; denominator via a
   ones-column matmul accumulated alongside PV; normalization folded into the
   PSUM->SBUF copy of O_T.
 - RoPE: rotate_half is a +-64 partition swap done with two SBUF DMAs; sin is
   pre-signed host-side, scale 1/sqrt(dh) pre-folded into Wq.
"""

import numpy as np
import ml_dtypes

import concourse.bass as bass
import concourse.mybir as mybir
from concourse import bacc
from concourse.tile import TileContext
from concourse.bass_utils import run_bass_kernel_spmd

BF16 = mybir.dt.bfloat16
F32 = mybir.dt.float32

B, S, H = 2, 2048, 4096
HEADS, DH = 32, 128
NCORES, TPDEG = 8, 4
HPC = HEADS // TPDEG          # heads per core = 8
GD = HPC * DH                 # group dim = 1024
NC32 = H // 128               # 32 contraction chunks for projections
NQS = S // 512                # 4 q-blocks of 512
NKT = S // 128                # 16 k-tiles of 128

LAST_RESULT = None            # BassKernelResults of the most recent run


def _build_program(causal: bool):
    nc = bacc.Bacc("TRN2", target_bir_lowering=False)

    xT = nc.dram_tensor("xT", [H, S], BF16, kind="ExternalInput")
    wqT = nc.dram_tensor("wqT", [H, GD], BF16, kind="ExternalInput")
    wkT = nc.dram_tensor("wkT", [H, GD], BF16, kind="ExternalInput")
    wvT = nc.dram_tensor("wvT", [H, GD], BF16, kind="ExternalInput")
    woT = nc.dram_tensor("woT", [GD, H], BF16, kind="ExternalInput")
    emT = nc.dram_tensor("emT", [S, S], BF16, kind="ExternalInput")  # exp(mask).T
    cosT = nc.dram_tensor("cosT", [DH, S], F32, kind="ExternalInput")
    sinT = nc.dram_tensor("sinT", [DH, S], F32, kind="ExternalInput")  # pre-signed
    y = nc.dram_tensor("y", [S, H], F32, kind="ExternalOutput")

    xT_r = xT.rearrange("(c p) q -> p c q", p=128)       # [128, 32, 2048]
    emT_r = emT.rearrange("(t p) q -> p t q", p=128)     # [128, 16, 2048]
    woT_r = woT.rearrange("(h p) j -> p h j", p=128)     # [128, 8, 4096]

    with TileContext(nc) as tc:
        from contextlib import ExitStack
        with ExitStack() as outer:
            cpool = outer.enter_context(tc.tile_pool(name="consts", bufs=1))
            pspool = outer.enter_context(
                tc.tile_pool(name="ps", bufs=8, space="PSUM"))

            cos_sb = cpool.tile([DH, S], F32, tag="cos")
            sin_sb = cpool.tile([DH, S], F32, tag="sin")
            nc.sync.dma_start(out=cos_sb, in_=cosT[:, :])
            nc.sync.dma_start(out=sin_sb, in_=sinT[:, :])
            ones_sb = cpool.tile([128, 1], BF16, tag="ones")
            nc.vector.memset(ones_sb, 1.0)

            qt_sb = cpool.tile([128, HPC, S], BF16, tag="qt")   # Q.T per head
            kt_sb = cpool.tile([128, HPC, S], BF16, tag="kt")   # K.T per head
            v_sb = cpool.tile([128, NKT, GD], BF16, tag="v")    # V natural

            # ---------------- Phase 1: QKV projections + RoPE ----------------
            with ExitStack() as ph1:
                xpool = ph1.enter_context(tc.tile_pool(name="x", bufs=1))
                wpool = ph1.enter_context(tc.tile_pool(name="w", bufs=4))
                spool = ph1.enter_context(tc.tile_pool(name="swp", bufs=2))
                tpool = ph1.enter_context(tc.tile_pool(name="tmp", bufs=2))

                for qs in range(NQS):
                    qsl = slice(qs * 512, (qs + 1) * 512)
                    xblk = xpool.tile([128, NC32, 512], BF16, tag="xblk")
                    nc.sync.dma_start(out=xblk, in_=xT_r[:, :, qsl])

                    for wdram, dest in ((wqT, qt_sb), (wkT, kt_sb)):
                        psums = [pspool.tile([128, 512], F32, tag="ps", name=f"pqk{qs}_{h}")
                                 for h in range(HPC)]
                        for c in range(NC32):
                            wc = wpool.tile([128, GD], BF16, tag="wc")
                            nc.sync.dma_start(
                                out=wc, in_=wdram[c * 128:(c + 1) * 128, :])
                            for h in range(HPC):
                                nc.tensor.matmul(
                                    psums[h],
                                    lhsT=wc[:, h * 128:(h + 1) * 128],
                                    rhs=xblk[:, c, :],
                                    start=(c == 0), stop=(c == NC32 - 1))
                        for h in range(HPC):
                            ps = psums[h]
                            ta = tpool.tile([128, 512], F32, tag="ta")
                            tb = tpool.tile([128, 512], F32, tag="tb")
                            nc.vector.tensor_mul(ta, ps, cos_sb[:, qsl])
                            # sin table is pre-signed for the post-swap slot, so
                            # multiply first, then partition-swap the product
                            nc.vector.tensor_mul(tb, ps, sin_sb[:, qsl])
                            swp = spool.tile([128, 512], F32, tag="swp")
                            nc.sync.dma_start(out=swp[0:64, :], in_=tb[64:128, :])
                            nc.sync.dma_start(out=swp[64:128, :], in_=tb[0:64, :])
                            nc.vector.tensor_add(dest[:, h, qsl], ta, swp)

                    # V: lhsT = x chunk (stationary), rhs = Wv.T chunk
                    psums = [pspool.tile([128, 512], F32, tag="ps", name=f"pv{qs}_{i}")
                             for i in range(8)]
                    for c in range(NC32):
                        wc = wpool.tile([128, GD], BF16, tag="wc")
                        nc.sync.dma_start(
                            out=wc, in_=wvT[c * 128:(c + 1) * 128, :])
                        for ktl in range(4):
                            for dh in range(2):
                                nc.tensor.matmul(
                                    psums[ktl * 2 + dh],
                                    lhsT=xblk[:, c, ktl * 128:(ktl + 1) * 128],
                                    rhs=wc[:, dh * 512:(dh + 1) * 512],
                                    start=(c == 0), stop=(c == NC32 - 1))
                    for ktl in range(4):
                        for dh in range(2):
                            nc.vector.tensor_copy(
                                out=v_sb[:, qs * 4 + ktl,
                                         dh * 512:(dh + 1) * 512],
                                in_=psums[ktl * 2 + dh])

            # ------------- Phase 2+3: attention + output projection -------------
            with ExitStack() as ph2:
                empool = ph2.enter_context(tc.tile_pool(name="em", bufs=1))
                ptpool = ph2.enter_context(tc.tile_pool(name="pt", bufs=4))
                pepool = ph2.enter_context(tc.tile_pool(name="pe", bufs=3))
                rcpool = ph2.enter_context(tc.tile_pool(name="rc", bufs=2))
                rbpool = ph2.enter_context(tc.tile_pool(name="rb", bufs=2))
                otpool = ph2.enter_context(tc.tile_pool(name="ot", bufs=2))
                wopool = ph2.enter_context(tc.tile_pool(name="wo", bufs=2))
                ypool = ph2.enter_context(tc.tile_pool(name="y", bufs=3))

                for qb in range(NQS):
                    qsl = slice(qb * 512, (qb + 1) * 512)
                    kt_hi = (qb + 1) * 4 if causal else NKT
                    diag_lo = qb * 4 if causal else 0
                    n_em = kt_hi - diag_lo
                    em_sb = empool.tile([128, n_em, 512], BF16, tag="em")
                    nc.sync.dma_start(
                        out=em_sb, in_=emT_r[:, diag_lo:kt_hi, qsl])

                    ot_qb = otpool.tile([128, HPC, 512], BF16, tag="ot")
                    for h in range(HPC):
                        o_ps = pspool.tile([128, 512], F32, tag="ps")
                        d_ps = pspool.tile([1, 512], F32, tag="ps")
                        for kt in range(kt_hi):
                            s_ps = pspool.tile([128, 512], F32, tag="ps")
                            nc.tensor.matmul(
                                s_ps,
                                lhsT=kt_sb[:, h, kt * 128:(kt + 1) * 128],
                                rhs=qt_sb[:, h, qsl],
                                start=True, stop=True)
                            pt = ptpool.tile([128, 512], BF16, tag="pt")
                            if kt >= diag_lo:
                                pe = pepool.tile([128, 512], BF16, tag="pe")
                                nc.scalar.activation(
                                    out=pe, in_=s_ps,
                                    func=mybir.ActivationFunctionType.Exp)
                                nc.vector.tensor_mul(
                                    pt, pe, em_sb[:, kt - diag_lo, :])
                            else:
                                nc.scalar.activation(
                                    out=pt, in_=s_ps,
                                    func=mybir.ActivationFunctionType.Exp)
                            nc.tensor.matmul(
                                o_ps,
                                lhsT=v_sb[:, kt, h * 128:(h + 1) * 128],
                                rhs=pt,
                                start=(kt == 0), stop=(kt == kt_hi - 1))
                            nc.tensor.matmul(
                                d_ps, lhsT=ones_sb, rhs=pt,
                                start=(kt == 0), stop=(kt == kt_hi - 1))
                        rc = rcpool.tile([1, 512], F32, tag="rc")
                        nc.vector.reciprocal(out=rc, in_=d_ps)
                        rb = rbpool.tile([128, 512], F32, tag="rb")
                        nc.gpsimd.partition_broadcast(rb, rc[:, :])
                        nc.vector.tensor_mul(ot_qb[:, h, :], o_ps, rb)

                    for jb in range(8):
                        jsl = slice(jb * 512, (jb + 1) * 512)
                        wo_sb = wopool.tile([128, HPC, 512], BF16, tag="wo")
                        nc.sync.dma_start(out=wo_sb, in_=woT_r[:, :, jsl])
                        for qt in range(4):
                            y_ps = pspool.tile([128, 512], F32, tag="ps")
                            for h in range(HPC):
                                nc.tensor.matmul(
                                    y_ps,
                                    lhsT=ot_qb[:, h, qt * 128:(qt + 1) * 128],
                                    rhs=wo_sb[:, h, :],
                                    start=(h == 0), stop=(h == HPC - 1))
                            ys = ypool.tile([128, 512], F32, tag="ys")
                            nc.vector.tensor_copy(out=ys, in_=y_ps)
                            nc.sync.dma_start(
                                out=y[qb * 512 + qt * 128:
                                      qb * 512 + (qt + 1) * 128, jsl],
                                in_=ys)

    nc.compile()
    return nc


_prog_cache = {}


def _get_program(causal: bool):
    if causal not in _prog_cache:
        _prog_cache[causal] = _build_program(causal)
    return _prog_cache[causal]


def kernel(hidden_states, Wq, Wk, Wv, Wo, attn_mask, position_ids):
    global LAST_RESULT
    hidden_states = np.asarray(hidden_states, dtype=np.float32)
    Wq = np.asarray(Wq, dtype=np.float32)
    Wk = np.asarray(Wk, dtype=np.float32)
    Wv = np.asarray(Wv, dtype=np.float32)
    Wo = np.asarray(Wo, dtype=np.float32)
    mask2d = np.asarray(attn_mask, dtype=np.float32).reshape(S, S)
    pos = np.asarray(position_ids).reshape(-1)[:S].astype(np.int64)

    bf = ml_dtypes.bfloat16

    # causal <=> strictly-upper entries fully masked, lower+diag entries 0
    tri = np.tril(np.ones((S, S), dtype=bool))
    causal = bool(np.all(mask2d[tri] == 0.0) and np.all(mask2d[~tri] < -1e30))

    # exp(mask), transposed: emT[k, q] = exp(mask[q, k])
    if causal:
        em = tri.astype(np.float32)
    else:
        em = np.exp(np.maximum(mask2d, -200.0))
    emT = np.ascontiguousarray(em.T).astype(bf)

    # RoPE tables (replicates reference.rope_cos_sin, indexed by position_ids)
    inv_freq = 1.0 / (10000.0 ** (np.arange(0, DH, 2, dtype=np.float64) / DH))
    t = pos.astype(np.float64)
    freqs = np.outer(t, inv_freq)                      # [S, 64]
    emb = np.concatenate([freqs, freqs], axis=-1)      # [S, 128]
    cos = np.cos(emb.astype(np.float32).astype(np.float64))
    sin = np.sin(emb.astype(np.float32).astype(np.float64))
    cosT = np.ascontiguousarray(cos.T).astype(np.float32)          # [128, S]
    sinT = np.ascontiguousarray(sin.T).astype(np.float32)
    # pre-signed for the post-swap slot: row d of the swapped product lands at
    # partition (d+64)%128, so negate the top half (see drain in _build_program)
    sinT[64:, :] *= -1.0

    scale = DH ** -0.5
    in_maps = []
    for c in range(NCORES):
        b, g = c // TPDEG, c % TPDEG
        sl = slice(g * GD, (g + 1) * GD)
        in_maps.append({
            "xT": np.ascontiguousarray(hidden_states[b].T).astype(bf),
            "wqT": np.ascontiguousarray((Wq[sl, :] * scale).T).astype(bf),
            "wkT": np.ascontiguousarray(Wk[sl, :].T).astype(bf),
            "wvT": np.ascontiguousarray(Wv[sl, :].T).astype(bf),
            "woT": np.ascontiguousarray(Wo[:, sl].T).astype(bf),
            "emT": emT,
            "cosT": cosT,
            "sinT": sinT,
        })

    nc = _get_program(causal)
    res = run_bass_kernel_spmd(nc, in_maps, core_ids=list(range(NCORES)))
    LAST_RESULT = res

    out = np.zeros((B, S, H), dtype=np.float32)
    for c in range(NCORES):
        out[c // TPDEG] += res.results[c]["y"]
    return out
